# revision 20
# baseline (speedup 1.0000x reference)
"""Trainium2 Bass kernel for nn_PointEncoder (PointNet-style encoder).

Data-parallel over 8 NeuronCores: 256 samples -> 32 per core.

Per-sample dataflow (points L=4096, hidden=64):
  h   = relu(Win @ xT + bin)                      [64, 4096]
  for i in 0..3:
      a    = relu(Li @ h + lbi)
      g    = max over points of a
      h    = relu(G1i @ a + G2i @ g + gbi)        (xs_i := h)
  out = max_l( sum_i Pi @ xs_i + pb )             [64]

On-chip layout: "stacked halves" — activations stored as [128, 2048] fp16
tiles: partitions 0-63 = hidden dims for points 0-2047, partitions 64-127 =
hidden dims for points 2048-4095.

Matmuls: 64x64 ops run as FOUR concurrent quadrant matmuls via
tile_position (the two diagonal 64x64 blocks of the block-diag const layout
are sliced as quadrant weights). Points migrate between halves across
layers (benign: the network is point-permutation invariant up to the final
max; all per-partition bias vectors are half-symmetric).

Drains: lyr PSUM is drained by a custom DVE op relu(psum + bias) that also
folds a running per-partition MAX into a second output — the global
max-pool rides the drain for free. glyr + proj_in drains run on the Scalar
engine (activation Relu with bias AP), on a SEPARATE psum pool so the two
drain engines stream concurrently. The tail projection PSUM is scanned by a
custom add-bias+max-accum DVE op, chained across chunks. The cross-half
maxes run as GpSimd software-DGE DMA pairs (copy + CCE max), keeping the
Vector queue clean.
"""
import sys
import numpy as np

sys.path.insert(0, "/opt/trn_rl_repo")

import concourse.bass as bass
import concourse.bacc as bacc
import concourse.mybir as mybir
from concourse import tile
from concourse.bass_utils import run_bass_kernel_spmd

F16 = mybir.dt.float16
F32 = mybir.dt.float32
AX = mybir.AluOpType
AF = mybir.ActivationFunctionType

N_CORES = 8
B_FULL = 256
NSAMP = B_FULL // N_CORES   # 32 samples per core
L = 4096                    # points per sample
H = 64                      # hidden
NL = 4                      # layers
LH = L // 2                 # 2048, stacked-half width
G = 5                       # samples per pipeline group

NEG = -3.0e38

# packed fp16 const layout (columns)
C16_WPI = 0          # [0:6, 0:128]
C16_WLYR = 128       # 4 x 128
C16_WGLYR = 640      # 4 x 128
C16_WPROJ = 1152     # 4 x 128
C16_COLS = 1664
# packed fp32 const layout (columns)
C32_WG2 = 0          # [0:64, 0:512], 4 x 128
C32_LBS = 512        # 4 (lyr_b stacked, per layer)
C32_GBS = 516        # 4 (glyr_b stacked, per layer)
C32_BPI = 520        # 1 (proj_in_b stacked)
C32_PBS = 521        # 1 (proj_out_b stacked)
C32_COLS = 523


# ---- custom DVE ops ------------------------------------------------------
# ANT_RELU_BIAS_MAXACC: out = relu(in + s0); accum_out = max-fold(out, init=s1)
# ANT_ADD_MAXACC:       out = in + s0;       accum_out = max-fold(out, init=s1)
def _register_dve_op(name, spec):
    from concourse import dve_ops as _dops
    from concourse.dve_spec import lower
    from concourse.dve_uop import DveOpSpec

    for op in _dops.OPS:
        if op.name == name:
            return op
    row = max(_dops._SUB_OPCODE_FOR_NAME.values()) + 1
    assert row < 0x20
    shas = {}
    for ver in ("v3", "v4"):
        s = DveOpSpec(name=name, opcode=row, uops=lower(spec, ver=ver),
                      rd1_en=False)
        shas[ver] = s.sha(ver)
    op = _dops.DveOp(name, spec, subdim=False, uops_sha=shas)
    _dops.OPS.append(op)
    _dops._SUB_OPCODE_FOR_NAME[name] = row
    _dops.CUSTOM_DVE_SPECS[name] = spec
    return op


def _make_ops():
    from concourse.dve_spec import Spec, Src0, C0, C1, relu, maxx
    return (
        _register_dve_op("ANT_RELU_BIAS_MAXACC",
                         Spec(body=relu(Src0 + C0), accum=maxx, accum_init=C1)),
        _register_dve_op("ANT_ADD_MAXACC",
                         Spec(body=Src0 + C0, accum=maxx, accum_init=C1)),
    )


RELU_MAXACC, ADD_MAXACC = _make_ops()


def build_nc(nsamp: int = NSAMP) -> bass.Bass:
    nc = bacc.Bacc()

    xT_d = nc.declare_dram_parameter("xT", [nsamp, 6, LH], F16, isOutput=False)
    c16_d = nc.declare_dram_parameter("c16", [128, C16_COLS], F16, isOutput=False)
    c32_d = nc.declare_dram_parameter("c32", [128, C32_COLS], F32, isOutput=False)
    out_d = nc.declare_dram_parameter("out", [nsamp, H], F32, isOutput=True)

    with tile.TileContext(nc) as tc:
        with (
            tc.tile_pool(name="consts", bufs=1) as cpool,
            tc.tile_pool(name="xin", bufs=G + 3) as xpool,
            tc.tile_pool(name="acts", bufs=G + 2) as hpool,
            tc.tile_pool(name="amid", bufs=G + 2) as apool,
            tc.tile_pool(name="xs", bufs=4 * G + 4) as xspool,
            tc.tile_pool(name="junk", bufs=2) as jpool,
            tc.tile_pool(name="tiny", bufs=16) as tpool,
            tc.tile_pool(name="ocoll", bufs=1) as opool,
            tc.tile_pool(name="plyr", bufs=3, space=bass.MemorySpace.PSUM) as plyr,
            tc.tile_pool(name="pglyr", bufs=2, space=bass.MemorySpace.PSUM) as pglyr,
        ):
            # ---- constants (two one-time DMAs) ----
            c16 = cpool.tile([128, C16_COLS], F16, tag="c16")
            nc.sync.dma_start(c16[:], c16_d[:])
            c32 = cpool.tile([128, C32_COLS], F32, tag="c32")
            nc.sync.dma_start(c32[:], c32_d[:])

            # Win.T replicated at 4 partition groups for 4-way projin MMs
            wpi4 = [c16[r : r + 3, 0:64] for r in (0, 32, 64, 96)]
            lbs = lambda i: c32[:, C32_LBS + i : C32_LBS + i + 1]
            gbs = lambda i: c32[:, C32_GBS + i : C32_GBS + i + 1]
            wg2 = lambda i: c32[0:64, C32_WG2 + 128 * i : C32_WG2 + 128 * i + 128]
            bpi = c32[:, C32_BPI : C32_BPI + 1]
            pbs_full = c32[:, C32_PBS : C32_PBS + 1]

            outcoll = opool.tile([64, nsamp], F32, tag="outc")

            def wblk(base, i):
                """(top, bottom) diagonal 64x64 blocks of weight i."""
                c = base + 128 * i
                return (c16[0:64, c : c + 64], c16[64:128, c + 64 : c + 128])

            def qmm4(ps, wt, wb, a, c0):
                """ps[128,1024] = W @ a[:, c0:c0+1024], 4 concurrent quadrants
                (one per PE-array 64x64 tile, all into one psum tile)."""
                nc.tensor.matmul(ps[0:64, 0:512], wt, a[0:64, c0 : c0 + 512],
                                 start=True, stop=True, tile_position=(0, 0),
                                 skip_group_check=True)
                nc.tensor.matmul(ps[64:128, 0:512], wb, a[64:128, c0 : c0 + 512],
                                 start=True, stop=True, tile_position=(64, 64))
                nc.tensor.matmul(ps[64:128, 512:1024], wt,
                                 a[0:64, c0 + 512 : c0 + 1024],
                                 start=True, stop=True, tile_position=(0, 64))
                nc.tensor.matmul(ps[0:64, 512:1024], wb,
                                 a[64:128, c0 + 512 : c0 + 1024],
                                 start=True, stop=True, tile_position=(64, 0))

            # ---- per-sample stage functions (st = in-flight state dict) ----
            def st_load(st):
                xt = xpool.tile([99, LH], F16, tag="xt", name=f"xt_{st['s']}")
                # half-A dims at rows 0-2 and 32-34, half-B at 64-66 and 96-98
                nc.sync.dma_start(xt[0:3, :], xT_d[st["s"], 0:3])
                nc.sync.dma_start(xt[32:35, :], xT_d[st["s"], 0:3])
                nc.sync.dma_start(xt[64:67, :], xT_d[st["s"], 3:6])
                nc.sync.dma_start(xt[96:99, :], xT_d[st["s"], 3:6])
                st["xt"] = xt

            def st_projin(st):
                h1 = hpool.tile([128, LH], F16, tag="h1")
                for t in range(2):
                    ps = plyr.tile([128, 1024], F32, tag="pl")
                    b = 1024 * t
                    xt = st["xt"]
                    nc.tensor.matmul(ps[0:64, 0:512], wpi4[0], xt[0:3, b : b + 512],
                                     start=True, stop=True, tile_position=(0, 0))
                    nc.tensor.matmul(ps[64:128, 0:512], wpi4[2],
                                     xt[64:67, b : b + 512],
                                     start=True, stop=True, tile_position=(64, 64))
                    nc.tensor.matmul(ps[64:128, 512:1024], wpi4[1],
                                     xt[32:35, b + 512 : b + 1024],
                                     start=True, stop=True, tile_position=(32, 64))
                    nc.tensor.matmul(ps[0:64, 512:1024], wpi4[3],
                                     xt[96:99, b + 512 : b + 1024],
                                     start=True, stop=True, tile_position=(96, 0))
                    nc.scalar.activation(
                        h1[:, 1024 * t : 1024 * t + 1024], ps[:], AF.Relu,
                        bias=bpi, scale=1.0,
                    )
                st["cur"] = h1
                st["xs"] = []

            def st_lyr(st, i):
                at = apool.tile([128, LH], F16, tag="at")
                wt, wb = wblk(C16_WLYR, i)
                m0 = tpool.tile([128, 1], F32, tag="m0")
                m1 = tpool.tile([128, 1], F32, tag="m1")
                pa = plyr.tile([128, 1024], F32, tag="pl", name="pa")
                qmm4(pa, wt, wb, st["cur"], 0)
                nc.vector._custom_dve(
                    RELU_MAXACC, out=at[:, 0:1024], in0=pa[:],
                    s0=lbs(i), s1=NEG, accum_out=m0[:, 0:1],
                )
                pb = plyr.tile([128, 1024], F32, tag="pl", name="pb")
                qmm4(pb, wt, wb, st["cur"], 1024)
                nc.vector._custom_dve(
                    RELU_MAXACC, out=at[:, 1024:2048], in0=pb[:],
                    s0=lbs(i), s1=m0[:, 0:1], accum_out=m1[:, 0:1],
                )
                st["at"] = at
                st["m"] = m1

            def st_pool(st, i):
                m = st["m"]
                mtop = tpool.tile([64, 1], F32, tag="mtop")
                nc.sync.dma_start(mtop[:], m[64:128, 0:1])
                gx = tpool.tile([64, 1], F32, tag="gx")
                nc.vector.tensor_max(gx[:], m[0:64, 0:1], mtop[:])
                # tiny matmul writes into the corner of the sample's upcoming
                # glyr psum tile (read by the v-add before glyr MMs overwrite)
                pv = pglyr.tile([128, 512], F32, tag="pg", name="ga")
                nc.tensor.matmul(pv[:, 0:1], wg2(i), gx[:], start=True, stop=True,
                                 skip_group_check=True)
                v = tpool.tile([128, 1], F32, tag="v")
                nc.vector.tensor_scalar_add(v[:], pv[:, 0:1], gbs(i))
                st["v"] = v
                st["ga"] = pv

            def st_glyr(st, i):
                xs_i = xspool.tile([128, LH], F16, tag="xs")
                wt, wb = wblk(C16_WGLYR, i)
                at = st["at"]
                v = st["v"]
                for r, c0 in enumerate((0, 1024)):
                    if r == 0:
                        g1 = st.pop("ga")
                    else:
                        g1 = pglyr.tile([128, 512], F32, tag="pg", name="g1")
                    g2 = pglyr.tile([128, 512], F32, tag="pg", name="g2")
                    nc.tensor.matmul(g1[0:64, :], wt, at[0:64, c0 : c0 + 512],
                                     start=True, stop=True, tile_position=(0, 0),
                                     skip_group_check=True)
                    nc.tensor.matmul(g1[64:128, :], wb, at[64:128, c0 : c0 + 512],
                                     start=True, stop=True,
                                     tile_position=(64, 64))
                    nc.tensor.matmul(g2[64:128, :], wt,
                                     at[0:64, c0 + 512 : c0 + 1024],
                                     start=True, stop=True, tile_position=(0, 64))
                    nc.tensor.matmul(g2[0:64, :], wb,
                                     at[64:128, c0 + 512 : c0 + 1024],
                                     start=True, stop=True, tile_position=(64, 0))
                    nc.scalar.activation(
                        xs_i[:, c0 : c0 + 512], g1[:], AF.Relu,
                        bias=v[:, 0:1], scale=1.0,
                    )
                    nc.scalar.activation(
                        xs_i[:, c0 + 512 : c0 + 1024], g2[:], AF.Relu,
                        bias=v[:, 0:1], scale=1.0,
                    )
                st["xs"].append(xs_i)
                st["cur"] = xs_i

            def st_tail(st):
                s = st["s"]
                macc = [tpool.tile([128, 1], F32, tag=f"ma{c}", name=f"ma{c}_{s}")
                        for c in range(2)]
                for cc in range(2):
                    base = 512 * cc
                    pt = plyr.tile([128, 1024], F32, tag="pl", name="pt")
                    # per piece: 4-concurrent quadrant MMs on this tile
                    for i in range(NL):
                        wt, wb = wblk(C16_WPROJ, i)
                        xs_i = st["xs"][i]
                        nc.tensor.matmul(
                            pt[0:64, 0:512], wt, xs_i[0:64, base : base + 512],
                            start=(i == 0), stop=(i == NL - 1),
                            tile_position=(0, 0),
                        )
                        nc.tensor.matmul(
                            pt[64:128, 0:512], wb,
                            xs_i[64:128, base : base + 512],
                            start=(i == 0), stop=(i == NL - 1),
                            tile_position=(64, 64),
                        )
                        nc.tensor.matmul(
                            pt[64:128, 512:1024], wt,
                            xs_i[0:64, base + 1024 : base + 1536],
                            start=(i == 0), stop=(i == NL - 1),
                            tile_position=(0, 64),
                        )
                        nc.tensor.matmul(
                            pt[0:64, 512:1024], wb,
                            xs_i[64:128, base + 1024 : base + 1536],
                            start=(i == 0), stop=(i == NL - 1),
                            tile_position=(64, 0),
                        )
                    junk = jpool.tile([128, 1024], F16, tag="junk")
                    nc.vector._custom_dve(
                        ADD_MAXACC, out=junk[:], in0=pt[:], s0=pbs_full,
                        s1=(NEG if cc == 0 else macc[0][:, 0:1]),
                        accum_out=macc[cc][:, 0:1],
                    )
                mproj = macc[1]
                mptop = tpool.tile([64, 1], F32, tag="mptop")
                nc.sync.dma_start(mptop[:], mproj[64:128, 0:1])
                nc.vector.tensor_max(
                    outcoll[:, s : s + 1], mproj[0:64, 0:1], mptop[:])

            # ---- software-pipelined emission -------------------------------
            # Within a layer, lyr(s_k) is emitted before pool+glyr(s_{k-1}) so
            # the in-order PE queue alternates DVE-drained and ACT-drained
            # matmul blocks (both drain engines stay fed) and the pool chain
            # latency of each sample hides behind the next sample's lyr MMs.
            # At group boundaries, tails interleave with the next group's
            # proj_in for the same reason.
            groups = [list(range(g, min(g + G, nsamp)))
                      for g in range(0, nsamp, G)]
            states = {}
            for s in groups[0]:
                states[s] = {"s": s}
                st_load(states[s])
            for st in [states[s] for s in groups[0]]:
                st_projin(st)
            for gi, grp in enumerate(groups):
                nxt = groups[gi + 1] if gi + 1 < len(groups) else []
                for s in nxt:
                    states[s] = {"s": s}
                    st_load(states[s])
                sts = [states[s] for s in grp]
                for i in range(NL):
                    for k, st in enumerate(sts):
                        st_lyr(st, i)
                        if k >= 1:
                            st_pool(sts[k - 1], i)
                            st_glyr(sts[k - 1], i)
                    st_pool(sts[-1], i)
                    st_glyr(sts[-1], i)
                nsts = [states[s] for s in nxt]
                for k, st in enumerate(sts):
                    st_tail(st)
                    if k < len(nsts):
                        st_projin(nsts[k])
                for st in nsts[len(sts):]:
                    st_projin(st)

            # ---- write output: out[s, e] = outcoll[e, s] ----
            nc.sync.dma_start(out_d[:].rearrange("s e -> e s"), outcoll[:])

    nc.finalize()
    return nc


def prep_maps(x: np.ndarray, proj_in_w, proj_in_b, lyr_w, lyr_b, glyr_w,
              glyr_b, proj_out_w, proj_out_b, nsamp: int = NSAMP,
              n_cores: int = N_CORES):
    """Host-side packing: transpose/cast x, build block-diag weight layouts."""
    B = x.shape[0]
    # [B,1,4096,3] -> [B, 2, 3, 2048] -> [B, 6, 2048] fp16
    xT = np.ascontiguousarray(
        x.reshape(B, 2, LH, 3).transpose(0, 1, 3, 2)
    ).reshape(B, 6, LH).astype(np.float16)

    def diag2(w):  # [64,64] -> [128,128] block-diag of w.T
        z = np.zeros((128, 128), np.float32)
        z[0:64, 0:64] = w.T
        z[64:128, 64:128] = w.T
        return z

    G1 = glyr_w[:, :, :H]           # (4,64,64)
    G2 = glyr_w[:, :, H:]           # (4,64,64)
    P = proj_out_w.reshape(H, NL, H).transpose(1, 0, 2)  # piece i: (64,64)

    c16 = np.zeros((128, C16_COLS), np.float32)
    for r in (0, 32, 64, 96):
        c16[r : r + 3, 0:64] = proj_in_w.T
    for i in range(NL):
        c16[:, C16_WLYR + 128 * i : C16_WLYR + 128 * (i + 1)] = diag2(lyr_w[i])
        c16[:, C16_WGLYR + 128 * i : C16_WGLYR + 128 * (i + 1)] = diag2(G1[i])
        c16[:, C16_WPROJ + 128 * i : C16_WPROJ + 128 * (i + 1)] = diag2(P[i])

    c32 = np.zeros((128, C32_COLS), np.float32)
    for i in range(NL):
        c32[0:64, C32_WG2 + 128 * i : C32_WG2 + 128 * i + 64] = G2[i].T
        c32[0:64, C32_WG2 + 128 * i + 64 : C32_WG2 + 128 * (i + 1)] = G2[i].T
        c32[:, C32_LBS + i] = np.tile(lyr_b[i], 2)
        c32[:, C32_GBS + i] = np.tile(glyr_b[i], 2)
    c32[:, C32_BPI] = np.tile(proj_in_b, 2)
    c32[:, C32_PBS] = np.tile(proj_out_b, 2)

    const_map = {
        "c16": c16.astype(np.float16),
        "c32": c32.astype(np.float32),
    }
    in_maps = []
    for ci in range(n_cores):
        m = dict(const_map)
        m["xT"] = np.ascontiguousarray(xT[ci * nsamp : (ci + 1) * nsamp])
        in_maps.append(m)
    return in_maps


_NC_CACHE = {}


def _get_nc(nsamp=NSAMP):
    if nsamp not in _NC_CACHE:
        _NC_CACHE[nsamp] = build_nc(nsamp)
    return _NC_CACHE[nsamp]


def kernel(x, proj_in_w, proj_in_b, lyr_w, lyr_b, glyr_w, glyr_b,
           proj_out_w, proj_out_b, _trace: bool = False):
    args = [np.asarray(a) for a in
            (x, proj_in_w, proj_in_b, lyr_w, lyr_b, glyr_w, glyr_b,
             proj_out_w, proj_out_b)]
    in_maps = prep_maps(*args)
    nc = _get_nc()
    res = run_bass_kernel_spmd(nc, in_maps, list(range(N_CORES)), trace=_trace)
    out = np.concatenate([r["out"] for r in res.results], 0).astype(np.float32)
    if _trace:
        return out, res
    return out


# revision 21
# speedup vs baseline: 1.0498x; 1.0498x over previous
"""Trainium2 Bass kernel for nn_PointEncoder (PointNet-style encoder).

Data-parallel over 8 NeuronCores: 256 samples -> 32 per core.

Per-sample dataflow (points L=4096, hidden=64):
  h   = relu(Win @ xT + bin)                      [64, 4096]
  for i in 0..3:
      a    = relu(Li @ h + lbi)
      g    = max over points of a
      h    = relu(G1i @ a + G2i @ g + gbi)        (xs_i := h)
  out = max_l( sum_i Pi @ xs_i + pb )             [64]

On-chip layout: "stacked halves" — activations stored as [128, 2048] fp16
tiles: partitions 0-63 = hidden dims for points 0-2047, partitions 64-127 =
hidden dims for points 2048-4095.

Matmuls: 64x64 ops run as FOUR concurrent quadrant matmuls via
tile_position (the two diagonal 64x64 blocks of the block-diag const layout
are sliced as quadrant weights). Points migrate between halves across
layers (benign: the network is point-permutation invariant up to the final
max; all per-partition bias vectors are half-symmetric).

Drains: lyr PSUM is drained by a custom DVE op relu(psum + bias) that also
folds a running per-partition MAX into a second output — the global
max-pool rides the drain for free. glyr + proj_in drains run on the Scalar
engine (activation Relu with bias AP), on a SEPARATE psum pool so the two
drain engines stream concurrently. The tail projection PSUM is scanned by a
custom add-bias+max-accum DVE op, chained across chunks. The cross-half
maxes run as GpSimd software-DGE DMA pairs (copy + CCE max), keeping the
Vector queue clean.
"""
import sys
import numpy as np

sys.path.insert(0, "/opt/trn_rl_repo")

import concourse.bass as bass
import concourse.bacc as bacc
import concourse.mybir as mybir
from concourse import tile
from concourse.bass_utils import run_bass_kernel_spmd

F16 = mybir.dt.float16
F32 = mybir.dt.float32
AX = mybir.AluOpType
AF = mybir.ActivationFunctionType

N_CORES = 8
B_FULL = 256
NSAMP = B_FULL // N_CORES   # 32 samples per core
L = 4096                    # points per sample
H = 64                      # hidden
NL = 4                      # layers
LH = L // 2                 # 2048, stacked-half width
G = 6                       # samples per pipeline group

NEG = -3.0e38

# packed fp16 const layout (columns)
C16_WPI = 0          # [0:6, 0:128]
C16_WLYR = 128       # 4 x 128
C16_WGLYR = 640      # 4 x 128
C16_WPROJ = 1152     # 4 x 128
C16_COLS = 1664
# packed fp32 const layout (columns)
C32_WG2 = 0          # [0:64, 0:512], 4 x 128
C32_LBS = 512        # 4 (lyr_b stacked, per layer)
C32_GBS = 516        # 4 (glyr_b stacked, per layer)
C32_BPI = 520        # 1 (proj_in_b stacked)
C32_PBS = 521        # 1 (proj_out_b stacked)
C32_COLS = 523


# ---- custom DVE ops ------------------------------------------------------
# ANT_RELU_BIAS_MAXACC: out = relu(in + s0); accum_out = max-fold(out, init=s1)
# ANT_ADD_MAXACC:       out = in + s0;       accum_out = max-fold(out, init=s1)
def _register_dve_op(name, spec):
    from concourse import dve_ops as _dops
    from concourse.dve_spec import lower
    from concourse.dve_uop import DveOpSpec

    for op in _dops.OPS:
        if op.name == name:
            return op
    row = max(_dops._SUB_OPCODE_FOR_NAME.values()) + 1
    assert row < 0x20
    shas = {}
    for ver in ("v3", "v4"):
        s = DveOpSpec(name=name, opcode=row, uops=lower(spec, ver=ver),
                      rd1_en=False)
        shas[ver] = s.sha(ver)
    op = _dops.DveOp(name, spec, subdim=False, uops_sha=shas)
    _dops.OPS.append(op)
    _dops._SUB_OPCODE_FOR_NAME[name] = row
    _dops.CUSTOM_DVE_SPECS[name] = spec
    return op


def _make_ops():
    from concourse.dve_spec import Spec, Src0, C0, C1, relu, maxx
    return (
        _register_dve_op("ANT_RELU_BIAS_MAXACC",
                         Spec(body=relu(Src0 + C0), accum=maxx, accum_init=C1)),
        _register_dve_op("ANT_ADD_MAXACC",
                         Spec(body=Src0 + C0, accum=maxx, accum_init=C1)),
    )


RELU_MAXACC, ADD_MAXACC = _make_ops()


def build_nc(nsamp: int = NSAMP) -> bass.Bass:
    nc = bacc.Bacc()

    xT_d = nc.declare_dram_parameter("xT", [nsamp, 6, LH], F16, isOutput=False)
    c16_d = nc.declare_dram_parameter("c16", [128, C16_COLS], F16, isOutput=False)
    c32_d = nc.declare_dram_parameter("c32", [128, C32_COLS], F32, isOutput=False)
    out_d = nc.declare_dram_parameter("out", [nsamp, H], F32, isOutput=True)

    with tile.TileContext(nc) as tc:
        with (
            tc.tile_pool(name="consts", bufs=1) as cpool,
            tc.tile_pool(name="xin", bufs=G + 2) as xpool,
            tc.tile_pool(name="acts", bufs=G + 1) as hpool,
            tc.tile_pool(name="amid", bufs=G + 1) as apool,
            tc.tile_pool(name="xs", bufs=4 * G + 2) as xspool,
            tc.tile_pool(name="junk", bufs=2) as jpool,
            tc.tile_pool(name="tiny", bufs=16) as tpool,
            tc.tile_pool(name="ocoll", bufs=1) as opool,
            tc.tile_pool(name="plyr", bufs=2, space=bass.MemorySpace.PSUM) as plyr,
            tc.tile_pool(name="pglyr", bufs=2, space=bass.MemorySpace.PSUM) as pglyr,
        ):
            # ---- constants (two one-time DMAs) ----
            c16 = cpool.tile([128, C16_COLS], F16, tag="c16")
            nc.sync.dma_start(c16[:], c16_d[:])
            c32 = cpool.tile([128, C32_COLS], F32, tag="c32")
            nc.sync.dma_start(c32[:], c32_d[:])

            # Win.T replicated at 4 partition groups for 4-way projin MMs
            wpi4 = [c16[r : r + 3, 0:64] for r in (0, 32, 64, 96)]
            lbs = lambda i: c32[:, C32_LBS + i : C32_LBS + i + 1]
            gbs = lambda i: c32[:, C32_GBS + i : C32_GBS + i + 1]
            wg2 = lambda i: c32[0:64, C32_WG2 + 128 * i : C32_WG2 + 128 * i + 128]
            bpi = c32[:, C32_BPI : C32_BPI + 1]
            pbs_full = c32[:, C32_PBS : C32_PBS + 1]

            outcoll = opool.tile([64, nsamp], F32, tag="outc")

            def wblk(base, i):
                """(top, bottom) diagonal 64x64 blocks of weight i."""
                c = base + 128 * i
                return (c16[0:64, c : c + 64], c16[64:128, c + 64 : c + 128])

            def qmm4(ps, wt, wb, a, c0):
                """ps[128,1024] = W @ a[:, c0:c0+1024], 4 concurrent quadrants
                (one per PE-array 64x64 tile, all into one psum tile)."""
                nc.tensor.matmul(ps[0:64, 0:512], wt, a[0:64, c0 : c0 + 512],
                                 start=True, stop=True, tile_position=(0, 0),
                                 skip_group_check=True)
                nc.tensor.matmul(ps[64:128, 0:512], wb, a[64:128, c0 : c0 + 512],
                                 start=True, stop=True, tile_position=(64, 64))
                nc.tensor.matmul(ps[64:128, 512:1024], wt,
                                 a[0:64, c0 + 512 : c0 + 1024],
                                 start=True, stop=True, tile_position=(0, 64))
                nc.tensor.matmul(ps[0:64, 512:1024], wb,
                                 a[64:128, c0 + 512 : c0 + 1024],
                                 start=True, stop=True, tile_position=(64, 0))

            # ---- per-sample stage functions (st = in-flight state dict) ----
            def st_load(st):
                xt = xpool.tile([99, LH], F16, tag="xt", name=f"xt_{st['s']}")
                # half-A dims at rows 0-2 and 32-34, half-B at 64-66 and 96-98
                nc.sync.dma_start(xt[0:3, :], xT_d[st["s"], 0:3])
                nc.sync.dma_start(xt[32:35, :], xT_d[st["s"], 0:3])
                nc.sync.dma_start(xt[64:67, :], xT_d[st["s"], 3:6])
                nc.sync.dma_start(xt[96:99, :], xT_d[st["s"], 3:6])
                st["xt"] = xt

            def st_projin(st):
                h1 = hpool.tile([128, LH], F16, tag="h1")
                for t in range(2):
                    ps = plyr.tile([128, 1024], F32, tag="pl")
                    b = 1024 * t
                    xt = st["xt"]
                    nc.tensor.matmul(ps[0:64, 0:512], wpi4[0], xt[0:3, b : b + 512],
                                     start=True, stop=True, tile_position=(0, 0))
                    nc.tensor.matmul(ps[64:128, 0:512], wpi4[2],
                                     xt[64:67, b : b + 512],
                                     start=True, stop=True, tile_position=(64, 64))
                    nc.tensor.matmul(ps[64:128, 512:1024], wpi4[1],
                                     xt[32:35, b + 512 : b + 1024],
                                     start=True, stop=True, tile_position=(32, 64))
                    nc.tensor.matmul(ps[0:64, 512:1024], wpi4[3],
                                     xt[96:99, b + 512 : b + 1024],
                                     start=True, stop=True, tile_position=(96, 0))
                    nc.scalar.activation(
                        h1[:, 1024 * t : 1024 * t + 1024], ps[:], AF.Relu,
                        bias=bpi, scale=1.0,
                    )
                st["cur"] = h1
                st["xs"] = []

            def st_lyr(st, i):
                at = apool.tile([128, LH], F16, tag="at")
                wt, wb = wblk(C16_WLYR, i)
                m0 = tpool.tile([128, 1], F32, tag="m0")
                m1 = tpool.tile([128, 1], F32, tag="m1")
                pa = plyr.tile([128, 1024], F32, tag="pl", name="pa")
                qmm4(pa, wt, wb, st["cur"], 0)
                nc.vector._custom_dve(
                    RELU_MAXACC, out=at[:, 0:1024], in0=pa[:],
                    s0=lbs(i), s1=NEG, accum_out=m0[:, 0:1],
                )
                pb = plyr.tile([128, 1024], F32, tag="pl", name="pb")
                qmm4(pb, wt, wb, st["cur"], 1024)
                nc.vector._custom_dve(
                    RELU_MAXACC, out=at[:, 1024:2048], in0=pb[:],
                    s0=lbs(i), s1=m0[:, 0:1], accum_out=m1[:, 0:1],
                )
                st["at"] = at
                st["m"] = m1

            def st_pool(st, i):
                m = st["m"]
                mtop = tpool.tile([64, 1], F32, tag="mtop")
                nc.sync.dma_start(mtop[:], m[64:128, 0:1])
                gx = tpool.tile([64, 1], F32, tag="gx")
                nc.vector.tensor_max(gx[:], m[0:64, 0:1], mtop[:])
                # tiny matmul writes into the corner of the sample's upcoming
                # glyr psum tile (read by the v-add before glyr MMs overwrite)
                pv = pglyr.tile([128, 1024], F32, tag="pg", name="ga")
                nc.tensor.matmul(pv[:, 0:1], wg2(i), gx[:], start=True, stop=True,
                                 skip_group_check=True)
                v = tpool.tile([128, 1], F32, tag="v")
                nc.vector.tensor_scalar_add(v[:], pv[:, 0:1], gbs(i))
                st["v"] = v
                st["ga"] = pv

            def st_glyr(st, i):
                xs_i = xspool.tile([128, LH], F16, tag="xs")
                wt, wb = wblk(C16_WGLYR, i)
                pa = st.pop("ga")
                qmm4(pa, wt, wb, st["at"], 0)
                nc.scalar.activation(
                    xs_i[:, 0:1024], pa[:], AF.Relu, bias=st["v"][:, 0:1],
                    scale=1.0,
                )
                pb = pglyr.tile([128, 1024], F32, tag="pg", name="gb")
                qmm4(pb, wt, wb, st["at"], 1024)
                nc.scalar.activation(
                    xs_i[:, 1024:2048], pb[:], AF.Relu, bias=st["v"][:, 0:1],
                    scale=1.0,
                )
                st["xs"].append(xs_i)
                st["cur"] = xs_i

            def st_tail(st):
                s = st["s"]
                macc = [tpool.tile([128, 1], F32, tag=f"ma{c}", name=f"ma{c}_{s}")
                        for c in range(2)]
                for cc in range(2):
                    base = 512 * cc
                    pt = pglyr.tile([128, 1024], F32, tag="pg", name="pt")
                    # per piece: 4-concurrent quadrant MMs on this tile
                    for i in range(NL):
                        wt, wb = wblk(C16_WPROJ, i)
                        xs_i = st["xs"][i]
                        nc.tensor.matmul(
                            pt[0:64, 0:512], wt, xs_i[0:64, base : base + 512],
                            start=(i == 0), stop=(i == NL - 1),
                            tile_position=(0, 0),
                        )
                        nc.tensor.matmul(
                            pt[64:128, 0:512], wb,
                            xs_i[64:128, base : base + 512],
                            start=(i == 0), stop=(i == NL - 1),
                            tile_position=(64, 64),
                        )
                        nc.tensor.matmul(
                            pt[64:128, 512:1024], wt,
                            xs_i[0:64, base + 1024 : base + 1536],
                            start=(i == 0), stop=(i == NL - 1),
                            tile_position=(0, 64),
                        )
                        nc.tensor.matmul(
                            pt[0:64, 512:1024], wb,
                            xs_i[64:128, base + 1024 : base + 1536],
                            start=(i == 0), stop=(i == NL - 1),
                            tile_position=(64, 0),
                        )
                    junk = jpool.tile([128, 1024], F16, tag="junk")
                    nc.vector._custom_dve(
                        ADD_MAXACC, out=junk[:], in0=pt[:], s0=pbs_full,
                        s1=(NEG if cc == 0 else macc[0][:, 0:1]),
                        accum_out=macc[cc][:, 0:1],
                    )
                mproj = macc[1]
                mptop = tpool.tile([64, 1], F32, tag="mptop")
                nc.sync.dma_start(mptop[:], mproj[64:128, 0:1])
                nc.vector.tensor_max(
                    outcoll[:, s : s + 1], mproj[0:64, 0:1], mptop[:])

            # ---- software-pipelined emission -------------------------------
            # Within a layer, lyr(s_k) is emitted before pool+glyr(s_{k-1}) so
            # the in-order PE queue alternates DVE-drained and ACT-drained
            # matmul blocks (both drain engines stay fed) and the pool chain
            # latency of each sample hides behind the next sample's lyr MMs.
            # At group boundaries, tails interleave with the next group's
            # proj_in for the same reason.
            groups = [list(range(g, min(g + G, nsamp)))
                      for g in range(0, nsamp, G)]
            states = {}
            for s in groups[0]:
                states[s] = {"s": s}
                st_load(states[s])
            for st in [states[s] for s in groups[0]]:
                st_projin(st)
            for gi, grp in enumerate(groups):
                nxt = groups[gi + 1] if gi + 1 < len(groups) else []
                for s in nxt:
                    states[s] = {"s": s}
                    st_load(states[s])
                sts = [states[s] for s in grp]
                for i in range(NL):
                    for k, st in enumerate(sts):
                        st_lyr(st, i)
                        if k >= 1:
                            st_pool(sts[k - 1], i)
                            st_glyr(sts[k - 1], i)
                    st_pool(sts[-1], i)
                    st_glyr(sts[-1], i)
                nsts = [states[s] for s in nxt]
                for k, st in enumerate(sts):
                    st_tail(st)
                    if k < len(nsts):
                        st_projin(nsts[k])
                for st in nsts[len(sts):]:
                    st_projin(st)

            # ---- write output: out[s, e] = outcoll[e, s] ----
            nc.sync.dma_start(out_d[:].rearrange("s e -> e s"), outcoll[:])

    nc.finalize()
    return nc


def prep_maps(x: np.ndarray, proj_in_w, proj_in_b, lyr_w, lyr_b, glyr_w,
              glyr_b, proj_out_w, proj_out_b, nsamp: int = NSAMP,
              n_cores: int = N_CORES):
    """Host-side packing: transpose/cast x, build block-diag weight layouts."""
    B = x.shape[0]
    # [B,1,4096,3] -> [B, 2, 3, 2048] -> [B, 6, 2048] fp16
    xT = np.ascontiguousarray(
        x.reshape(B, 2, LH, 3).transpose(0, 1, 3, 2)
    ).reshape(B, 6, LH).astype(np.float16)

    def diag2(w):  # [64,64] -> [128,128] block-diag of w.T
        z = np.zeros((128, 128), np.float32)
        z[0:64, 0:64] = w.T
        z[64:128, 64:128] = w.T
        return z

    G1 = glyr_w[:, :, :H]           # (4,64,64)
    G2 = glyr_w[:, :, H:]           # (4,64,64)
    P = proj_out_w.reshape(H, NL, H).transpose(1, 0, 2)  # piece i: (64,64)

    c16 = np.zeros((128, C16_COLS), np.float32)
    for r in (0, 32, 64, 96):
        c16[r : r + 3, 0:64] = proj_in_w.T
    for i in range(NL):
        c16[:, C16_WLYR + 128 * i : C16_WLYR + 128 * (i + 1)] = diag2(lyr_w[i])
        c16[:, C16_WGLYR + 128 * i : C16_WGLYR + 128 * (i + 1)] = diag2(G1[i])
        c16[:, C16_WPROJ + 128 * i : C16_WPROJ + 128 * (i + 1)] = diag2(P[i])

    c32 = np.zeros((128, C32_COLS), np.float32)
    for i in range(NL):
        c32[0:64, C32_WG2 + 128 * i : C32_WG2 + 128 * i + 64] = G2[i].T
        c32[0:64, C32_WG2 + 128 * i + 64 : C32_WG2 + 128 * (i + 1)] = G2[i].T
        c32[:, C32_LBS + i] = np.tile(lyr_b[i], 2)
        c32[:, C32_GBS + i] = np.tile(glyr_b[i], 2)
    c32[:, C32_BPI] = np.tile(proj_in_b, 2)
    c32[:, C32_PBS] = np.tile(proj_out_b, 2)

    const_map = {
        "c16": c16.astype(np.float16),
        "c32": c32.astype(np.float32),
    }
    in_maps = []
    for ci in range(n_cores):
        m = dict(const_map)
        m["xT"] = np.ascontiguousarray(xT[ci * nsamp : (ci + 1) * nsamp])
        in_maps.append(m)
    return in_maps


_NC_CACHE = {}


def _get_nc(nsamp=NSAMP):
    if nsamp not in _NC_CACHE:
        _NC_CACHE[nsamp] = build_nc(nsamp)
    return _NC_CACHE[nsamp]


def kernel(x, proj_in_w, proj_in_b, lyr_w, lyr_b, glyr_w, glyr_b,
           proj_out_w, proj_out_b, _trace: bool = False):
    args = [np.asarray(a) for a in
            (x, proj_in_w, proj_in_b, lyr_w, lyr_b, glyr_w, glyr_b,
             proj_out_w, proj_out_b)]
    in_maps = prep_maps(*args)
    nc = _get_nc()
    res = run_bass_kernel_spmd(nc, in_maps, list(range(N_CORES)), trace=_trace)
    out = np.concatenate([r["out"] for r in res.results], 0).astype(np.float32)
    if _trace:
        return out, res
    return out
